# revision 10
# baseline (speedup 1.0000x reference)
"""BigBird block Trainium2 kernel: 8-core SPMD, v2.

Sharding: core c -> batch b = c//4, group rank g = c%4.
  - attention: heads 4g..4g+3 (C=256 qkv cols), tensor-parallel
  - Wo partials ReduceScattered over token chunks: rank g receives the
    group-summed attention output for tokens [512g, 512(g+1))
  - FFN: token-parallel with the FULL 4096 hidden dim per core; each
    core emits the final x2+ff for its own 512 tokens (no second
    collective, no host-side partial summing)

All compute uses feature-major ("transposed") layouts [feature, token] so
matmul contractions keep features on partitions.  LN1 is folded into the
QKV projections (per-token mu/rsig applied post-matmul); LN2 is explicit.
Softmax denominators ride along the attn@V matmul via a ones column
appended to each V tile (65-wide per-head stationary operand).
q/k/attn-out/Wo/W1/W2 run in bf16 (PSUM accumulation stays f32).
"""
import sys
from contextlib import ExitStack

sys.path.insert(0, "/opt/trn_rl_repo")
import numpy as np
import concourse.bacc as bacc
import concourse.mybir as mybir
from concourse import tile

F32 = mybir.dt.float32
F32R = mybir.dt.float32r
BF16 = mybir.dt.bfloat16
NPBF16 = mybir.dt.np(BF16)

B, T, D, H, HD = 2, 2048, 1024, 16, 64
C = 256          # qkv cols per core (4 heads)
FF = 4096        # full ffn hidden (token-parallel ffn)
TC = 512         # tokens per core after reduce-scatter
NCORES = 8
GROUPS = [[0, 1, 2, 3], [4, 5, 6, 7]]
DT8 = D // 128   # 8 d-tiles
TT16 = T // 128  # 16 token tiles
HM32 = FF // 128  # 32 hidden tiles
LN_EPS = 1e-5

AF = mybir.ActivationFunctionType
OP = mybir.AluOpType


def r32(ap):
    return ap.bitcast(F32R)


def build_nc():
    nc = bacc.Bacc("TRN2", target_bir_lowering=False, debug=False,
                   num_devices=NCORES)
    dt = nc.dram_tensor
    xT = dt("xT", [D, T], F32R, kind="ExternalInput")
    xTc = dt("xTc", [D, TC], F32, kind="ExternalInput")
    maskT = dt("maskT", [T, T], BF16, kind="ExternalInput")
    wq = dt("wq", [D, C], F32R, kind="ExternalInput")
    wk = dt("wk", [D, C], F32R, kind="ExternalInput")
    wv = dt("wv", [D, C], F32R, kind="ExternalInput")
    wo = dt("wo", [C, D], BF16, kind="ExternalInput")
    w1 = dt("w1", [D, FF], BF16, kind="ExternalInput")
    w2 = dt("w2", [FF, D], BF16, kind="ExternalInput")
    wsq = dt("wsq", [128, 2], F32, kind="ExternalInput")   # colsum of wq
    wsk = dt("wsk", [128, 2], F32, kind="ExternalInput")
    wsv_bc = dt("wsv_bc", [128, C], F32, kind="ExternalInput")
    bq = dt("bq", [128, 2], F32, kind="ExternalInput")     # ln1_b @ Wq
    bk = dt("bk", [128, 2], F32, kind="ExternalInput")
    bv_bc = dt("bv_bc", [128, C], F32, kind="ExternalInput")
    bo_col = dt("bo_col", [128, DT8], F32, kind="ExternalInput")
    b1_col = dt("b1_col", [128, HM32], F32, kind="ExternalInput")
    ws1_col = dt("ws1_col", [128, HM32], F32, kind="ExternalInput")
    b2_col = dt("b2_col", [128, DT8], F32, kind="ExternalInput")

    xout = dt("xoutT", [D, TC], F32, kind="ExternalOutput")
    ar_in = dt("ar_in", [4, D, TC], BF16, kind="Internal")
    ar_out = dt("ar_out", [D, TC], BF16, kind="Internal")

    with ExitStack() as es:
        es.enter_context(nc.allow_low_precision(
            reason="bf16/fp32r SBUF tiles feed the PE; accumulation stays f32"))
        tc = es.enter_context(tile.TileContext(nc))

        def pool(name, bufs, space="SBUF"):
            return tc.tile_pool(name=name, bufs=bufs, space=space)

        pp = es.enter_context(pool("persist", 1))
        ones_sb = pp.tile([128, 128], F32R, name="ones_sb")
        nc.gpsimd.memset(ones_sb[:].bitcast(F32), 1.0)
        inv128 = pp.tile([128, 1], F32R, name="inv128")
        nc.gpsimd.memset(inv128[:].bitcast(F32), 1.0 / 128.0)

        # qT/kT/v live from phase 2 through phase 4
        s234 = es.enter_context(ExitStack())
        qsb = s234.enter_context(pool("qkv_sb", 1))

        # ================ phases 1+2: LN1 stats + QKV ====================
        with ExitStack() as s12:
            xres = s12.enter_context(pool("xres", 1))
            xts = []
            for d in range(DT8):
                t_ = xres.tile([128, T], F32R, tag=f"xt{d}", name=f"xt{d}")
                nc.sync.dma_start(t_[:], xT[d * 128:(d + 1) * 128, :])
                xts.append(t_)

            statsb = s12.enter_context(pool("statsb", 1))
            mu_bc = statsb.tile([128, T], F32, tag="mu", name="mu")
            nrsig_bc = statsb.tile([128, T], F32, tag="nrsig", name="nrsig")
            murs_bc = statsb.tile([128, T], F32, tag="murs", name="murs")
            wrk = statsb.tile([128, T], F32, tag="wrk", name="wrk")

            with pool("sqp", 2) as sqp, pool("statps", 1, "PSUM") as statps:
                sum_ps = [statps.tile([128, 512], F32, tag=f"sum{n}", name=f"sum{n}")
                          for n in range(4)]
                sq_ps = [statps.tile([128, 512], F32, tag=f"sq{n}", name=f"sq{n}")
                         for n in range(4)]
                for d in range(DT8):
                    sq = sqp.tile([128, T], F32R, tag="sq", name="sq")
                    nc.scalar.activation(sq[:], xts[d][:], AF.Square)
                    for n in range(4):
                        sl = slice(n * 512, (n + 1) * 512)
                        nc.tensor.matmul(sum_ps[n][:], r32(ones_sb[:]),
                                         r32(xts[d][:, sl]),
                                         start=(d == 0), stop=(d == DT8 - 1),
                                         skip_group_check=True)
                        nc.tensor.matmul(sq_ps[n][:], r32(ones_sb[:]),
                                         r32(sq[:, sl]),
                                         start=(d == 0), stop=(d == DT8 - 1),
                                         skip_group_check=True)
                for n in range(4):
                    sl = slice(n * 512, (n + 1) * 512)
                    nc.vector.tensor_scalar_mul(mu_bc[:, sl], sum_ps[n][:],
                                                1.0 / D)
                    nc.vector.tensor_scalar_mul(wrk[:, sl], sq_ps[n][:],
                                                1.0 / D)
            # var = E[x^2] - mu^2 + eps; rsig = 1/sqrt(var)
            nc.vector.tensor_tensor(murs_bc[:], mu_bc[:], mu_bc[:], OP.mult)
            nc.vector.tensor_sub(wrk[:], wrk[:], murs_bc[:])
            nc.vector.tensor_scalar_add(wrk[:], wrk[:], LN_EPS)
            nc.scalar.activation(wrk[:], wrk[:], AF.Sqrt)
            nc.vector.reciprocal(murs_bc[:], wrk[:])          # rsig (temp)
            nc.vector.tensor_scalar_mul(nrsig_bc[:], murs_bc[:], -1.0)
            nc.vector.tensor_tensor(murs_bc[:], mu_bc[:], murs_bc[:],
                                    OP.mult)                  # mu*rsig

            # per-token scalar columns (for the v path)
            rsig_col, murs_col = [], []
            with pool("colps", 2, "PSUM") as cps:
                for tt in range(TT16):
                    sl = slice(tt * 128, (tt + 1) * 128)
                    pr = cps.tile([128, 1], F32, tag="pr", name="pr")
                    nc.tensor.matmul(pr[:], nrsig_bc[:, sl],
                                     inv128[:].bitcast(F32),
                                     start=True, stop=True,
                                     skip_group_check=True)
                    rc = statsb.tile([128, 1], F32, tag=f"rc{tt}", name=f"rc{tt}")
                    nc.vector.tensor_scalar_mul(rc[:], pr[:], -1.0)
                    rsig_col.append(rc)
                    pm = cps.tile([128, 1], F32, tag="pm", name="pm")
                    nc.tensor.matmul(pm[:], murs_bc[:, sl],
                                     inv128[:].bitcast(F32),
                                     start=True, stop=True,
                                     skip_group_check=True)
                    mc = statsb.tile([128, 1], F32, tag=f"mc{tt}", name=f"mc{tt}")
                    nc.vector.tensor_copy(mc[:], pm[:])
                    murs_col.append(mc)

            # ---- QKV ----
            with pool("wqkv", 1) as wp, pool("qkps", 4, "PSUM") as qkps, \
                 pool("qtmp", 2) as qtmp:
                wq_sb, wk_sb, wv_sb = [], [], []
                for d in range(DT8):
                    for nm, dr, lst in (("wq", wq, wq_sb), ("wk", wk, wk_sb),
                                        ("wv", wv, wv_sb)):
                        w_ = wp.tile([128, C], F32R, tag=f"{nm}{d}", name=f"{nm}{d}")
                        nc.sync.dma_start(w_[:],
                                          dr[d * 128:(d + 1) * 128, :])
                        lst.append(w_)
                scal = {}
                for nm, dr in (("wsq", wsq), ("wsk", wsk), ("bq", bq),
                               ("bk", bk)):
                    s_ = wp.tile([128, 2], F32, tag=nm, name=nm)
                    nc.sync.dma_start(s_[:], dr[:])
                    scal[nm] = s_
                wsv_sb = wp.tile([128, C], F32, tag="wsv", name="wsv")
                nc.sync.dma_start(wsv_sb[:], wsv_bc[:])
                bv_sb = wp.tile([128, C], F32, tag="bv", name="bv")
                nc.sync.dma_start(bv_sb[:], bv_bc[:])

                qT, kT = [], []
                for zname, wz, lst, ws_key, b_key in (
                        ("q", wq_sb, qT, "wsq", "bq"),
                        ("k", wk_sb, kT, "wsk", "bk")):
                    for m in range(2):
                        zt = qsb.tile([128, T], BF16, tag=f"{zname}T{m}", name=f"{zname}T{m}")
                        msl = slice(m * 128, (m + 1) * 128)
                        for n in range(4):
                            nsl = slice(n * 512, (n + 1) * 512)
                            zp = qkps.tile([128, 512], F32, tag="zp", name="zp")
                            for d in range(DT8):
                                nc.tensor.matmul(
                                    zp[:], r32(wz[d][:, msl]),
                                    r32(xts[d][:, nsl]),
                                    start=(d == 0), stop=(d == DT8 - 1),
                                    skip_group_check=True)
                            # (mu*wsz - raw); then z = that*(-rsig) + b
                            tmpz = qtmp.tile([128, 512], F32, tag="tmpz", name="tmpz")
                            nc.vector.scalar_tensor_tensor(
                                tmpz[:], mu_bc[:, nsl],
                                scal[ws_key][:, m:m + 1], zp[:],
                                OP.mult, OP.subtract)
                            nc.vector.tensor_tensor(tmpz[:], tmpz[:],
                                                    nrsig_bc[:, nsl], OP.mult)
                            nc.vector.tensor_scalar(zt[:, nsl], tmpz[:],
                                                    scal[b_key][:, m:m + 1],
                                                    None, OP.add)
                        lst.append(zt)

                # v natural [t-part, c-free] bf16, 65-stride + ones column
                v_sb = []
                for tt in range(TT16):
                    vt = qsb.tile([128, 4 * 65], BF16, tag=f"v{tt}", name=f"v{tt}")
                    v3 = vt[:].rearrange("p (h c) -> p h c", h=4)
                    nc.gpsimd.memset(v3[:, :, 64:65], 1.0)
                    vp = qkps.tile([128, C], F32, tag="vp", name="vp")
                    tsl = slice(tt * 128, (tt + 1) * 128)
                    for d in range(DT8):
                        nc.tensor.matmul(vp[:], r32(xts[d][:, tsl]),
                                         r32(wv_sb[d][:]),
                                         start=(d == 0), stop=(d == DT8 - 1),
                                         skip_group_check=True)
                    tmp2 = qtmp.tile([128, C], F32, tag="tmp2", name="tmp2")
                    nc.vector.tensor_scalar(tmp2[:], wsv_sb[:],
                                            murs_col[tt][:], None, OP.mult)
                    nc.vector.tensor_sub(tmp2[:], bv_sb[:], tmp2[:])
                    vp3 = vp[:].rearrange("p (h c) -> p h c", h=4)
                    t23 = tmp2[:].rearrange("p (h c) -> p h c", h=4)
                    nc.vector.scalar_tensor_tensor(
                        v3[:, :, 0:64], vp3[:, :, :], rsig_col[tt][:],
                        t23[:, :, :], OP.mult, OP.add)
                    v_sb.append(vt)
        # s12 closed: xts + stats freed; qT/kT/v_sb persist in qsb.

        # ================ phase 3+4: attention, Wo, ReduceScatter ========
        asb = s234.enter_context(pool("att_sb", 1))
        attnT = [asb.tile([128, T], BF16, tag=f"aT{m}", name=f"aT{m}") for m in range(2)]
        den = [asb.tile([1, T], F32, tag=f"den{h}", name=f"den{h}")
               for h in range(4)]
        rden = [asb.tile([1, T], F32R, tag=f"rden{h}", name=f"rden{h}")
                for h in range(4)]
        wo_sb = []
        for cc in range(2):
            w_ = asb.tile([128, D], BF16, tag=f"wo{cc}", name=f"wo{cc}")
            nc.sync.dma_start(w_[:], wo[cc * 128:(cc + 1) * 128, :])
            wo_sb.append(w_)
        with pool("mskp", 1) as mskp, pool("ptp", 4) as ptp, \
             pool("sps", 2, "PSUM") as spsp, \
             pool("avps", 1, "PSUM") as avps, \
             pool("dnps", 1, "PSUM") as dnps, \
             pool("arp", 2) as arp:
            for j in range(4):
                n_kt = 4 * j + 4
                qsl = slice(j * 512, (j + 1) * 512)
                mts = {}
                for kt in range(n_kt):
                    ksl = slice(kt * 128, (kt + 1) * 128)
                    mt = mskp.tile([128, 512], BF16, tag=f"mt{kt}",
                                   name=f"mt{kt}")
                    nc.sync.dma_start(mt[:], maskT[ksl, qsl])
                    mts[kt] = mt
                for hp in range(2):
                    avA = avps.tile([65, 512], F32, tag="avA", name="avA")
                    avB = avps.tile([65, 512], F32, tag="avB", name="avB")
                    for kt in range(n_kt):
                        ksl = slice(kt * 128, (kt + 1) * 128)
                        sps = spsp.tile([128, 1024], F32, tag="sps", name="sps")
                        nc.tensor.matmul(
                            sps[:, 0:512], kT[hp][0:64, ksl],
                            qT[hp][0:64, qsl], start=True, stop=True,
                            tile_position=(0, 0), skip_group_check=True)
                        nc.tensor.matmul(
                            sps[:, 512:1024], kT[hp][64:128, ksl],
                            qT[hp][64:128, qsl], start=True, stop=True,
                            tile_position=(64, 0), skip_group_check=True)
                        pt = ptp.tile([128, 1024], BF16, tag="pt", name="pt")
                        nc.scalar.activation(pt[:], sps[:], AF.Exp,
                                             scale=0.125)
                        nc.vector.tensor_mul(pt[:, 0:512], pt[:, 0:512],
                                             mts[kt][:])
                        nc.vector.tensor_mul(pt[:, 512:1024],
                                             pt[:, 512:1024], mts[kt][:])
                        vv = v_sb[kt][:].rearrange("p (h c) -> p h c", h=4)
                        nc.tensor.matmul(
                            avA[:], vv[:, 2 * hp, :], pt[:, 0:512],
                            start=(kt == 0), stop=(kt == n_kt - 1),
                            skip_group_check=True)
                        nc.tensor.matmul(
                            avB[:], vv[:, 2 * hp + 1, :], pt[:, 512:1024],
                            start=(kt == 0), stop=(kt == n_kt - 1),
                            skip_group_check=True)
                    nc.scalar.copy(attnT[hp][0:64, qsl], avA[0:64, :])
                    nc.scalar.copy(attnT[hp][64:128, qsl], avB[0:64, :])
                    nc.vector.tensor_copy(den[2 * hp][:, qsl], avA[64:65, :])
                    nc.vector.tensor_copy(den[2 * hp + 1][:, qsl],
                                          avB[64:65, :])
                # normalize chunk j and project through Wo, then RS input j
                for h in range(4):
                    nc.vector.reciprocal(rden[h][:, qsl], den[h][:, qsl])
                for m in range(2):
                    dp = dnps.tile([128, 512], F32, tag="dp", name="dp")
                    nc.tensor.matmul(dp[0:64, :],
                                     ones_sb[0:1, 0:64].bitcast(F32),
                                     rden[2 * m][:, qsl].bitcast(F32),
                                     start=True, stop=True,
                                     skip_group_check=True)
                    nc.tensor.matmul(dp[64:128, :],
                                     ones_sb[0:1, 64:128].bitcast(F32),
                                     rden[2 * m + 1][:, qsl].bitcast(F32),
                                     start=True, stop=True,
                                     skip_group_check=True)
                    nc.vector.tensor_mul(attnT[m][:, qsl], attnT[m][:, qsl],
                                         dp[:])
                for o in range(DT8):
                    osl = slice(o * 128, (o + 1) * 128)
                    wps = dnps.tile([128, 512], F32, tag="wps", name="wps")
                    for cc in range(2):
                        nc.tensor.matmul(
                            wps[:], wo_sb[cc][:, osl],
                            attnT[cc][:, qsl],
                            start=(cc == 0), stop=(cc == 1),
                            skip_group_check=True)
                    ao = arp.tile([128, 512], BF16, tag="ao", name="ao")
                    nc.scalar.copy(ao[:], wps[:])
                    nc.sync.dma_start(ar_in[j, osl, :], ao[:])
            # one ReduceScatter over token chunks: rank g gets chunk g summed
            nc.gpsimd.collective_compute(
                "ReduceScatter", mybir.AluOpType.add,
                replica_groups=GROUPS,
                ins=[ar_in[:]], outs=[ar_out[:]])
        s234.close()  # free qT/kT/v/attnT SBUF before phases 5-6

        # ========= phases 5+6: own-chunk x2 + LN2 + full-hidden FFN ======
        with pool("x2p", 1) as x2p, pool("ln2sb", 1) as ln2sb, \
             pool("sqp2", 2) as sqp2, pool("arl", 2) as arl, \
             pool("lnps", 2, "PSUM") as lnps, \
             pool("f1ps", 2, "PSUM") as f1ps, \
             pool("f2ps", 2, "PSUM") as f2ps, \
             pool("w1p", 2) as w1p, pool("a1p", 1) as a1p, \
             pool("xop", 2) as xop, pool("colp", 1) as colp, \
             pool("w2p", 1) as w2p:
            bo_sb = colp.tile([128, DT8], F32, tag="bo", name="bo")
            nc.sync.dma_start(bo_sb[:], bo_col[:])
            b1_sb = colp.tile([128, HM32], F32, tag="b1", name="b1")
            nc.sync.dma_start(b1_sb[:], b1_col[:])
            ws1_sb = colp.tile([128, HM32], F32, tag="ws1", name="ws1")
            nc.sync.dma_start(ws1_sb[:], ws1_col[:])
            b2_sb = colp.tile([128, DT8], F32, tag="b2", name="b2")
            nc.sync.dma_start(b2_sb[:], b2_col[:])
            w2_sb = []
            for hm in range(HM32):
                w2t = w2p.tile([128, D], BF16, tag=f"w2_{hm}", name=f"w2_{hm}")
                nc.sync.dma_start(w2t[:], w2[hm * 128:(hm + 1) * 128, :])
                w2_sb.append(w2t)

            # x2 = x + attn_out + bo for own tokens
            x2 = []
            for d in range(DT8):
                dsl = slice(d * 128, (d + 1) * 128)
                xt2 = x2p.tile([128, TC], F32R, tag=f"x2_{d}", name=f"x2_{d}")
                xr = arl.tile([128, TC], F32, tag="xr", name="xr")
                nc.sync.dma_start(xr[:], xTc[dsl, :])
                ar_t = arl.tile([128, TC], BF16, tag="art", name="art")
                nc.sync.dma_start(ar_t[:], ar_out[dsl, :])
                nc.vector.scalar_tensor_tensor(
                    xt2[:], ar_t[:], bo_sb[:, d:d + 1], xr[:],
                    OP.add, OP.add)
                x2.append(xt2)
            # LN2 stats for own chunk
            sum_ps = lnps.tile([128, TC], F32, tag="s2", name="s2")
            sq_ps = lnps.tile([128, TC], F32, tag="q2", name="q2")
            for d in range(DT8):
                sq = sqp2.tile([128, TC], F32R, tag="sq2", name="sq2")
                nc.scalar.activation(sq[:], x2[d][:], AF.Square)
                nc.tensor.matmul(sum_ps[:], r32(ones_sb[:]),
                                 r32(x2[d][:]),
                                 start=(d == 0), stop=(d == DT8 - 1),
                                 skip_group_check=True)
                nc.tensor.matmul(sq_ps[:], r32(ones_sb[:]), r32(sq[:]),
                                 start=(d == 0), stop=(d == DT8 - 1),
                                 skip_group_check=True)
            mu2 = ln2sb.tile([128, TC], F32, tag="mu2", name="mu2")
            nrsig2 = ln2sb.tile([128, TC], F32, tag="nrsig2", name="nrsig2")
            wrk2 = ln2sb.tile([128, TC], F32, tag="wrk2", name="wrk2")
            nc.vector.tensor_scalar_mul(mu2[:], sum_ps[:], 1.0 / D)
            nc.vector.tensor_scalar_mul(wrk2[:], sq_ps[:], 1.0 / D)
            nc.vector.tensor_tensor(nrsig2[:], mu2[:], mu2[:], OP.mult)
            nc.vector.tensor_sub(wrk2[:], wrk2[:], nrsig2[:])
            nc.vector.tensor_scalar_add(wrk2[:], wrk2[:], LN_EPS)
            nc.scalar.activation(wrk2[:], wrk2[:], AF.Sqrt)
            nc.vector.reciprocal(nrsig2[:], wrk2[:])
            murs2 = ln2sb.tile([128, TC], F32, tag="murs2", name="murs2")
            nc.vector.tensor_tensor(murs2[:], mu2[:], nrsig2[:], OP.mult)
            # x2s = x2 * rsig2 (bf16); mean correction folded via stt + gelu
            x2s = []
            for d in range(DT8):
                xs_ = ln2sb.tile([128, TC], BF16, tag=f"x2s_{d}",
                                 name=f"x2s_{d}")
                nc.vector.tensor_tensor(xs_[:], x2[d][:], nrsig2[:], OP.mult)
                x2s.append(xs_)
            # FFN1: stream w1 tiles by hidden-group; full 4096 hidden
            a1 = [a1p.tile([128, TC], BF16, tag=f"a1_{hm}", name=f"a1_{hm}")
                  for hm in range(HM32)]
            for hg in range(FF // 512):
                w1t = []
                for d in range(DT8):
                    dsl = slice(d * 128, (d + 1) * 128)
                    wt = w1p.tile([128, 512], BF16, tag=f"w1_{d}",
                                  name=f"w1_{d}")
                    nc.sync.dma_start(wt[:],
                                      w1[dsl, hg * 512:(hg + 1) * 512])
                    w1t.append(wt)
                for hl in range(4):
                    hm = hg * 4 + hl
                    hsl = slice(hl * 128, (hl + 1) * 128)
                    ap_ = f1ps.tile([128, TC], F32, tag="a1ps", name="a1ps")
                    for d in range(DT8):
                        nc.tensor.matmul(ap_[:], w1t[d][:, hsl], x2s[d][:],
                                         start=(d == 0), stop=(d == DT8 - 1),
                                         skip_group_check=True)
                    a1n = sqp2.tile([128, TC], F32, tag="a1n", name="a1n")
                    nc.vector.scalar_tensor_tensor(
                        a1n[:], murs2[:], ws1_sb[:, hm:hm + 1], ap_[:],
                        OP.mult, OP.subtract)
                    nc.scalar.activation(a1[hm][:], a1n[:], AF.Gelu,
                                         bias=b1_sb[:, hm:hm + 1], scale=-1.0)
            # FFN2 + residual + b2 -> final output for own tokens
            for om in range(DT8):
                osl = slice(om * 128, (om + 1) * 128)
                fp_ = f2ps.tile([128, TC], F32, tag="ffps", name="ffps")
                for hm in range(HM32):
                    nc.tensor.matmul(fp_[:], w2_sb[hm][:, osl], a1[hm][:],
                                     start=(hm == 0), stop=(hm == HM32 - 1),
                                     skip_group_check=True)
                xo = xop.tile([128, TC], F32, tag="xo", name="xo")
                nc.vector.scalar_tensor_tensor(
                    xo[:], fp_[:], b2_sb[:, om:om + 1], x2[om][:],
                    OP.add, OP.add)
                nc.sync.dma_start(xout[osl, :], xo[:])
    nc.compile()
    return nc


def host_prep(inputs):
    """Build per-core input maps from the full problem inputs."""
    x = np.asarray(inputs["x"], np.float32)
    mask = np.asarray(inputs["mask"])
    ln1_g = np.asarray(inputs["ln1_g"], np.float32)
    ln1_b = np.asarray(inputs["ln1_b"], np.float32)
    ln2_g = np.asarray(inputs["ln2_g"], np.float32)
    ln2_b = np.asarray(inputs["ln2_b"], np.float32)
    Wq = np.asarray(inputs["Wq"], np.float32)
    Wk = np.asarray(inputs["Wk"], np.float32)
    Wv = np.asarray(inputs["Wv"], np.float32)
    Wo = np.asarray(inputs["Wo"], np.float32)
    bo = np.asarray(inputs["bo"], np.float32)
    W1 = np.asarray(inputs["W1"], np.float32)
    b1 = np.asarray(inputs["b1"], np.float32)
    W2 = np.asarray(inputs["W2"], np.float32)
    b2 = np.asarray(inputs["b2"], np.float32)

    maskT = np.ascontiguousarray(mask.T).astype(np.float32).astype(NPBF16)
    Wq_f = ln1_g[:, None] * Wq
    Wk_f = ln1_g[:, None] * Wk
    Wv_f = ln1_g[:, None] * Wv
    W1_f = ln2_g[:, None] * W1
    w1_bf = W1_f.astype(NPBF16)
    w2_bf = np.ascontiguousarray(W2).astype(NPBF16)
    ws1 = w1_bf.astype(np.float32).sum(0)
    b1_full = ln2_b @ w1_bf.astype(np.float32) + b1
    in_maps = []
    for c in range(NCORES):
        b, g = divmod(c, 4)
        cs = slice(g * C, (g + 1) * C)
        wq_s = np.ascontiguousarray(Wq_f[:, cs])
        wk_s = np.ascontiguousarray(Wk_f[:, cs])
        wv_s = np.ascontiguousarray(Wv_f[:, cs])
        xTb = np.ascontiguousarray(x[b].T)
        m = {
            "xT": xTb,
            "xTc": np.ascontiguousarray(xTb[:, g * TC:(g + 1) * TC]),
            "maskT": maskT,
            "wq": wq_s, "wk": wk_s, "wv": wv_s,
            "wo": np.ascontiguousarray(Wo[cs, :]).astype(NPBF16),
            "w1": w1_bf,
            "w2": w2_bf,
            "wsq": wq_s.sum(0).reshape(2, 128).T.copy(),
            "wsk": wk_s.sum(0).reshape(2, 128).T.copy(),
            "wsv_bc": np.broadcast_to(wv_s.sum(0), (128, C)).copy(),
            "bq": (ln1_b @ Wq[:, cs]).reshape(2, 128).T.copy(),
            "bk": (ln1_b @ Wk[:, cs]).reshape(2, 128).T.copy(),
            "bv_bc": np.broadcast_to(ln1_b @ Wv[:, cs], (128, C)).copy(),
            "bo_col": bo.reshape(DT8, 128).T.copy(),
            "b1_col": b1_full.reshape(HM32, 128).T.copy(),
            "ws1_col": ws1.reshape(HM32, 128).T.copy(),
            "b2_col": b2.reshape(DT8, 128).T.copy(),
        }
        in_maps.append(m)
    return in_maps, b2


def host_assemble(out_maps, b2):
    out = np.empty((B, T, D), np.float32)
    for c in range(NCORES):
        b, g = divmod(c, 4)
        out[b, g * TC:(g + 1) * TC, :] = out_maps[c]["xoutT"].T
    return out


# ======================================================================
# Harness entry point
# ======================================================================
_NC_CACHE = {}


def _get_nc():
    if "nc" not in _NC_CACHE:
        _NC_CACHE["nc"] = build_nc()
    return _NC_CACHE["nc"]


def kernel(**inputs):
    """Full-input / full-output BigBird block on 8 NeuronCores."""
    from concourse.bass_utils import run_bass_kernel_spmd
    nc = _get_nc()
    in_maps, b2 = host_prep(inputs)
    res = run_bass_kernel_spmd(nc, in_maps, list(range(NCORES)))
    return host_assemble(res.results, b2)


# revision 57
# speedup vs baseline: 1.4659x; 1.4659x over previous
"""BigBird block Trainium2 kernel: 8-core SPMD, v2.

Sharding: core c -> batch b = c//4, group rank g = c%4.
  - attention: heads 4g..4g+3 (C=256 qkv cols), tensor-parallel
  - Wo partials ReduceScattered over token chunks: rank g receives the
    group-summed attention output for tokens [512g, 512(g+1))
  - FFN: token-parallel with the FULL 4096 hidden dim per core; each
    core emits the final x2+ff for its own 512 tokens (no second
    collective, no host-side partial summing)

All compute uses feature-major ("transposed") layouts [feature, token] so
matmul contractions keep features on partitions.  LN1 is folded into the
QKV projections (per-token mu/rsig applied post-matmul); LN2 is explicit.
Softmax denominators ride along the attn@V matmul via a ones column
appended to each V tile (65-wide per-head stationary operand).
q/k/attn-out/Wo/W1/W2 run in bf16 (PSUM accumulation stays f32).
"""
import sys
from contextlib import ExitStack

sys.path.insert(0, "/opt/trn_rl_repo")
import numpy as np
import concourse.bacc as bacc
import concourse.mybir as mybir
from concourse import tile

F32 = mybir.dt.float32
F32R = mybir.dt.float32r
BF16 = mybir.dt.bfloat16
FP8 = mybir.dt.float8e4
NPBF16 = mybir.dt.np(BF16)
NPFP8 = mybir.dt.np(FP8)
W8SCALE = 64.0   # fp8 weight pre-scale (keeps 0.02-scale weights normal)

B, T, D, H, HD = 2, 2048, 1024, 16, 64
C = 256          # qkv cols per core (4 heads)
FF = 4096        # full ffn hidden (token-parallel ffn)
TC = 512         # tokens per core after reduce-scatter
NCORES = 8
GROUPS = [[0, 1, 2, 3], [4, 5, 6, 7]]
DT8 = D // 128   # 8 d-tiles
TT16 = T // 128  # 16 token tiles
HM32 = FF // 128  # 32 hidden tiles
LN_EPS = 1e-5

AF = mybir.ActivationFunctionType
OP = mybir.AluOpType


def r32(ap):
    return ap.bitcast(F32R)


def osl_o(om):
    return slice(om * 128, (om + 1) * 128)


def build_nc():
    nc = bacc.Bacc("TRN2", target_bir_lowering=False, debug=False,
                   num_devices=NCORES)
    dt = nc.dram_tensor
    xT = dt("xT", [D, T], BF16, kind="ExternalInput")
    xTc = dt("xTc", [D, TC], F32, kind="ExternalInput")
    maskT = dt("maskT", [T, T], BF16, kind="ExternalInput")
    wqkv = dt("wqkv", [D, 3 * C], BF16, kind="ExternalInput")  # wq|wk|wv
    wo = dt("wo", [C, D], BF16, kind="ExternalInput")
    w1 = dt("w1", [D, FF], BF16, kind="ExternalInput")
    w2 = dt("w2", [FF, D], BF16, kind="ExternalInput")
    # scal blob: wsq|wsk|bq|bk (2 cols each) then wsv_bc|bv_bc (C each)
    scal_b = dt("scal_b", [128, 8 + 2 * C], F32, kind="ExternalInput")
    # col blob: bo (DT8) | b1 (HM32) | ws1 (HM32)
    col_b = dt("col_b", [128, DT8 + 2 * HM32], F32, kind="ExternalInput")

    xout = dt("xoutT", [D, TC], F32, kind="ExternalOutput")
    ar_in = dt("ar_in", [4, D, TC], BF16, kind="Internal")
    ar_out = dt("ar_out", [D, TC], BF16, kind="Internal")

    with ExitStack() as es:
        es.enter_context(nc.allow_low_precision(
            reason="bf16/fp32r SBUF tiles feed the PE; accumulation stays f32"))
        tc = es.enter_context(tile.TileContext(nc))

        def pool(name, bufs, space="SBUF"):
            return tc.tile_pool(name=name, bufs=bufs, space=space)

        pp = es.enter_context(pool("persist", 1))
        ones_sb = pp.tile([128, 128], F32R, name="ones_sb")
        nc.gpsimd.memset(ones_sb[:].bitcast(F32), 1.0)
        ones_sbf = pp.tile([128, 128], BF16, name="ones_sbf")
        nc.gpsimd.memset(ones_sbf[:], 1.0)
        inv128 = pp.tile([128, 1], F32R, name="inv128")
        nc.gpsimd.memset(inv128[:].bitcast(F32), 1.0 / 128.0)
        ones_bf = pp.tile([1, 128], BF16, name="ones_bf")
        nc.gpsimd.memset(ones_bf[:], 1.0)

        # qT/kT/v live from phase 2 through phase 4
        s234 = es.enter_context(ExitStack())
        qsb = s234.enter_context(pool("qkv_sb", 1))

        # ================ phases 1+2: LN1 stats + QKV ====================
        with ExitStack() as s12:
            # qkv weights + scalars first: batched small DMAs that unblock
            # the QKV matmuls; x tiles follow on the shared DMA device.
            wp = s12.enter_context(pool("wqkv", 1))
            wq_sb, wk_sb, wv_sb = [], [], []
            for d in range(DT8):
                w_ = wp.tile([128, 3 * C], BF16, tag=f"wqkv{d}",
                             name=f"wqkv{d}")
                nc.sync.dma_start(w_[:], wqkv[d * 128:(d + 1) * 128, :])
                wq_sb.append(w_[:, 0:C])
                wk_sb.append(w_[:, C:2 * C])
                wv_sb.append(w_[:, 2 * C:3 * C])
            scalt = wp.tile([128, 8 + 2 * C], F32, tag="scal", name="scal")
            nc.sync.dma_start(scalt[:], scal_b[:])
            scal = {"wsq": scalt[:, 0:2], "wsk": scalt[:, 2:4],
                    "bq": scalt[:, 4:6], "bk": scalt[:, 6:8]}
            wsv_sb = scalt[:, 8:8 + C]
            bv_sb = scalt[:, 8 + C:8 + 2 * C]

            xres = s12.enter_context(pool("xres", 1))
            xts = []
            for d in range(DT8):
                t_ = xres.tile([128, T], BF16, tag=f"xt{d}", name=f"xt{d}")
                nc.sync.dma_start(t_[:], xT[d * 128:(d + 1) * 128, :])
                xts.append(t_)

            statsb = s12.enter_context(pool("statsb", 1))
            mu_bc = statsb.tile([128, T], F32, tag="mu", name="mu")
            nrsig_bc = statsb.tile([128, T], F32, tag="nrsig", name="nrsig")
            murs_bc = statsb.tile([128, T], F32, tag="murs", name="murs")
            wrk = statsb.tile([128, T], F32, tag="wrk", name="wrk")

            # LN1 stats, chunked by 512-token groups so downstream work can
            # start as soon as the first chunk's mu/rsig are ready.
            rsig_col, murs_col = [None] * TT16, [None] * TT16
            with pool("sqp", 2) as sqp, pool("statps", 1, "PSUM") as statps:
                sum_ps = [statps.tile([128, 512], F32, tag=f"sum{n}", name=f"sum{n}")
                          for n in range(4)]
                sq_ps = [statps.tile([128, 512], F32, tag=f"sq{n}", name=f"sq{n}")
                         for n in range(4)]
                for d in range(DT8):
                    sq = sqp.tile([128, T], BF16, tag="sq", name="sq")
                    nc.scalar.activation(sq[:], xts[d][:], AF.Square)
                    for n in range(4):
                        sl = slice(n * 512, (n + 1) * 512)
                        nc.tensor.matmul(sum_ps[n][:], ones_sbf[:],
                                         xts[d][:, sl],
                                         start=(d == 0), stop=(d == DT8 - 1),
                                         skip_group_check=True)
                        nc.tensor.matmul(sq_ps[n][:], ones_sbf[:],
                                         sq[:, sl],
                                         start=(d == 0), stop=(d == DT8 - 1),
                                         skip_group_check=True)
                for n in range(4):
                    sl = slice(n * 512, (n + 1) * 512)
                    nc.vector.tensor_scalar_mul(mu_bc[:, sl], sum_ps[n][:],
                                                1.0 / D)
                    nc.vector.tensor_scalar_mul(wrk[:, sl], sq_ps[n][:],
                                                1.0 / D)
                    # var = E[x^2] - mu^2 + eps; rsig = 1/sqrt(var)
                    nc.vector.tensor_tensor(murs_bc[:, sl], mu_bc[:, sl],
                                            mu_bc[:, sl], OP.mult)
                    nc.vector.tensor_sub(wrk[:, sl], wrk[:, sl],
                                         murs_bc[:, sl])
                    nc.vector.tensor_scalar_add(wrk[:, sl], wrk[:, sl],
                                                LN_EPS)
                    nc.scalar.activation(wrk[:, sl], wrk[:, sl], AF.Sqrt)
                    nc.vector.reciprocal(murs_bc[:, sl], wrk[:, sl])  # rsig
                    nc.vector.tensor_scalar_mul(nrsig_bc[:, sl],
                                                murs_bc[:, sl], -1.0)
                    nc.vector.tensor_tensor(murs_bc[:, sl], mu_bc[:, sl],
                                            murs_bc[:, sl], OP.mult)
                    # per-token scalar columns (for the v path); reuse the
                    # consumed stats PSUM banks for the 1-col matmuls
                    for ti, tt in enumerate(range(4 * n, 4 * n + 4)):
                        tsl = slice(tt * 128, (tt + 1) * 128)
                        pr = sum_ps[n][:, ti:ti + 1]
                        nc.tensor.matmul(pr, nrsig_bc[:, tsl],
                                         inv128[:].bitcast(F32),
                                         start=True, stop=True,
                                         skip_group_check=True)
                        rc = statsb.tile([128, 1], F32, tag=f"rc{tt}",
                                         name=f"rc{tt}")
                        nc.vector.tensor_scalar_mul(rc[:], pr, -1.0)
                        rsig_col[tt] = rc
                        pm = sq_ps[n][:, ti:ti + 1]
                        nc.tensor.matmul(pm, murs_bc[:, tsl],
                                         inv128[:].bitcast(F32),
                                         start=True, stop=True,
                                         skip_group_check=True)
                        mc = statsb.tile([128, 1], F32, tag=f"mc{tt}",
                                         name=f"mc{tt}")
                        nc.vector.tensor_copy(mc[:], pm)
                        murs_col[tt] = mc

            # ---- QKV ----
            with pool("qkps", 4, "PSUM") as qkps, pool("qtmp", 2) as qtmp:
                qT, kT = [], []
                for zname, wz, lst, ws_key, b_key in (
                        ("q", wq_sb, qT, "wsq", "bq"),
                        ("k", wk_sb, kT, "wsk", "bk")):
                    for m in range(2):
                        zt = qsb.tile([128, T], BF16, tag=f"{zname}T{m}", name=f"{zname}T{m}")
                        msl = slice(m * 128, (m + 1) * 128)
                        for n in range(4):
                            nsl = slice(n * 512, (n + 1) * 512)
                            zp = qkps.tile([128, 512], F32, tag="zp", name="zp")
                            for d in range(DT8):
                                nc.tensor.matmul(
                                    zp[:], wz[d][:, msl],
                                    xts[d][:, nsl],
                                    start=(d == 0), stop=(d == DT8 - 1),
                                    skip_group_check=True)
                            # (mu*wsz - raw); then z = that*(-rsig) + b
                            tmpz = qtmp.tile([128, 512], F32, tag="tmpz", name="tmpz")
                            nc.vector.scalar_tensor_tensor(
                                tmpz[:], mu_bc[:, nsl],
                                scal[ws_key][:, m:m + 1], zp[:],
                                OP.mult, OP.subtract)
                            nc.vector.tensor_tensor(tmpz[:], tmpz[:],
                                                    nrsig_bc[:, nsl], OP.mult)
                            nc.vector.tensor_scalar(zt[:, nsl], tmpz[:],
                                                    scal[b_key][:, m:m + 1],
                                                    None, OP.add)
                        lst.append(zt)

                # v natural [t-part, c-free] bf16, 65-stride + ones column
                v_sb = []
                for tt in range(TT16):
                    vt = qsb.tile([128, 4 * 65], BF16, tag=f"v{tt}", name=f"v{tt}")
                    v3 = vt[:].rearrange("p (h c) -> p h c", h=4)
                    nc.gpsimd.memset(v3[:, :, 64:65], 1.0)
                    vp = qkps.tile([128, C], F32, tag="vp", name="vp")
                    tsl = slice(tt * 128, (tt + 1) * 128)
                    for d in range(DT8):
                        nc.tensor.matmul(vp[:], xts[d][:, tsl],
                                         wv_sb[d][:],
                                         start=(d == 0), stop=(d == DT8 - 1),
                                         skip_group_check=True)
                    tmp2 = qtmp.tile([128, C], F32, tag="tmp2", name="tmp2")
                    nc.vector.tensor_scalar(tmp2[:], wsv_sb[:],
                                            murs_col[tt][:], None, OP.mult)
                    nc.vector.tensor_sub(tmp2[:], bv_sb[:], tmp2[:])
                    vp3 = vp[:].rearrange("p (h c) -> p h c", h=4)
                    t23 = tmp2[:].rearrange("p (h c) -> p h c", h=4)
                    nc.vector.scalar_tensor_tensor(
                        v3[:, :, 0:64], vp3[:, :, :], rsig_col[tt][:],
                        t23[:, :, :], OP.mult, OP.add)
                    v_sb.append(vt)
        # s12 closed: xts + stats freed; qT/kT/v_sb persist in qsb.

        # ================ phase 3+4: attention, Wo, ReduceScatter ========
        asb = s234.enter_context(pool("att_sb", 1))
        attnT = [asb.tile([128, T], BF16, tag=f"aT{m}", name=f"aT{m}") for m in range(2)]
        den = [asb.tile([1, T], F32, tag=f"den{h}", name=f"den{h}")
               for h in range(4)]
        rden = [asb.tile([1, T], BF16, tag=f"rden{h}", name=f"rden{h}")
                for h in range(4)]
        wo_sb = []
        for cc in range(2):
            w_ = asb.tile([128, D], BF16, tag=f"wo{cc}", name=f"wo{cc}")
            nc.sync.dma_start(w_[:], wo[cc * 128:(cc + 1) * 128, :])
            wo_sb.append(w_)
        with pool("mskp", 1) as mskp, pool("ptp", 4) as ptp, \
             pool("sps", 2, "PSUM") as spsp, \
             pool("avps", 1, "PSUM") as avps, \
             pool("dnps", 1, "PSUM") as dnps, \
             pool("arp", 2) as arp:
            for j in range(4):
                n_kt = 4 * j + 4
                qsl = slice(j * 512, (j + 1) * 512)
                # one batched mask DMA per chunk: [p, kt, q] <- maskT rows
                # (single rotating tag keeps far-future chunks from
                # hoisting their loads ahead of the critical x tiles)
                mj = mskp.tile([128, n_kt, 512], BF16, tag="mj",
                               name=f"mj{j}")
                nc.sync.dma_start(
                    mj[:], maskT[0:n_kt * 128, qsl]
                    .rearrange("(kt p) q -> p kt q", p=128))
                mts = [mj[:, kt, :] for kt in range(n_kt)]
                for hp in range(2):
                    avA = avps.tile([65, 512], F32, tag="avA", name="avA")
                    avB = avps.tile([65, 512], F32, tag="avB", name="avB")
                    for kt in range(n_kt):
                        ksl = slice(kt * 128, (kt + 1) * 128)
                        sps = spsp.tile([128, 1024], F32, tag="sps", name="sps")
                        nc.tensor.matmul(
                            sps[:, 0:512], kT[hp][0:64, ksl],
                            qT[hp][0:64, qsl], start=True, stop=True,
                            tile_position=(0, 0), skip_group_check=True)
                        nc.tensor.matmul(
                            sps[:, 512:1024], kT[hp][64:128, ksl],
                            qT[hp][64:128, qsl], start=True, stop=True,
                            tile_position=(64, 0), skip_group_check=True)
                        pt = ptp.tile([128, 1024], BF16, tag="pt", name="pt")
                        nc.scalar.activation(pt[:], sps[:], AF.Exp,
                                             scale=0.125)
                        # mask multiply: offload 1/3 to the idle gpsimd
                        meng = nc.gpsimd if kt % 3 == 2 else nc.vector
                        meng.tensor_mul(pt[:, 0:512], pt[:, 0:512],
                                        mts[kt][:])
                        meng.tensor_mul(pt[:, 512:1024],
                                        pt[:, 512:1024], mts[kt][:])
                        vv = v_sb[kt][:].rearrange("p (h c) -> p h c", h=4)
                        nc.tensor.matmul(
                            avA[:], vv[:, 2 * hp, :], pt[:, 0:512],
                            start=(kt == 0), stop=(kt == n_kt - 1),
                            skip_group_check=True)
                        nc.tensor.matmul(
                            avB[:], vv[:, 2 * hp + 1, :], pt[:, 512:1024],
                            start=(kt == 0), stop=(kt == n_kt - 1),
                            skip_group_check=True)
                    nc.scalar.copy(attnT[hp][0:64, qsl], avA[0:64, :])
                    nc.scalar.copy(attnT[hp][64:128, qsl], avB[0:64, :])
                    nc.vector.tensor_copy(den[2 * hp][:, qsl], avA[64:65, :])
                    nc.vector.tensor_copy(den[2 * hp + 1][:, qsl],
                                          avB[64:65, :])
                # normalize chunk j and project through Wo, then RS input j
                for h in range(4):
                    nc.vector.reciprocal(rden[h][:, qsl], den[h][:, qsl])
                for m in range(2):
                    dp = dnps.tile([128, 512], F32, tag="dp", name="dp")
                    nc.tensor.matmul(dp[0:64, :],
                                     ones_bf[0:1, 0:64],
                                     rden[2 * m][:, qsl],
                                     start=True, stop=True,
                                     skip_group_check=True)
                    nc.tensor.matmul(dp[64:128, :],
                                     ones_bf[0:1, 64:128],
                                     rden[2 * m + 1][:, qsl],
                                     start=True, stop=True,
                                     skip_group_check=True)
                    nc.vector.tensor_mul(attnT[m][:, qsl], attnT[m][:, qsl],
                                         dp[:])
                aoj = arp.tile([128, DT8, 512], BF16, tag="ao", name="ao")
                for o in range(DT8):
                    osl = slice(o * 128, (o + 1) * 128)
                    wps = dnps.tile([128, 512], F32, tag="wps", name="wps")
                    for cc in range(2):
                        nc.tensor.matmul(
                            wps[:], wo_sb[cc][:, osl],
                            attnT[cc][:, qsl],
                            start=(cc == 0), stop=(cc == 1),
                            skip_group_check=True)
                    nc.vector.tensor_copy(aoj[:, o, :], wps[:])
                nc.sync.dma_start(
                    ar_in[j, :, :].rearrange("(o p) q -> p o q", p=128),
                    aoj[:])
            # one ReduceScatter over token chunks: rank g gets chunk g summed
            nc.gpsimd.collective_compute(
                "ReduceScatter", mybir.AluOpType.add,
                replica_groups=GROUPS,
                ins=[ar_in[:]], outs=[ar_out[:]])
        s234.close()  # free qT/kT/v/attnT SBUF before phases 5-6

        # ========= phases 5+6: own-chunk x2 + LN2 + full-hidden FFN ======
        with pool("x2p", 1) as x2p, pool("ln2sb", 1) as ln2sb, \
             pool("sqp2", 2) as sqp2, pool("arl", 1) as arl, \
             pool("lnps", 1, "PSUM") as lnps, \
             pool("f1ps", 2, "PSUM") as f1ps, \
             pool("f2ps", 1, "PSUM") as f2ps, \
             pool("w1p", 2) as w1p, pool("a1p", 1) as a1p, \
             pool("xop", 1) as xop, pool("colp", 1) as colp, \
             pool("w2p", 1) as w2p:
            colt = colp.tile([128, DT8 + 2 * HM32], F32, tag="colb",
                             name="colb")
            nc.sync.dma_start(colt[:], col_b[:])
            bo_sb = colt[:, 0:DT8]
            b1_sb = colt[:, DT8:DT8 + HM32]
            ws1_sb = colt[:, DT8 + HM32:DT8 + 2 * HM32]
            w2_sb = []
            for h4 in range(HM32 // 4):
                w2t = w2p.tile([128, 4, D], BF16, tag=f"w2_{h4}",
                               name=f"w2_{h4}")
                nc.sync.dma_start(
                    w2t[:], w2[h4 * 512:(h4 + 1) * 512, :]
                    .rearrange("(hh p) o -> p hh o", p=128))
                w2_sb.append(w2t)

            # x2 = x + attn_out + bo for own tokens (batched loads)
            xr8 = arl.tile([128, DT8, TC], F32, tag="xr", name="xr")
            nc.sync.dma_start(
                xr8[:], xTc[:].rearrange("(d p) q -> p d q", p=128))
            ar8 = arl.tile([128, DT8, TC], BF16, tag="art", name="art")
            nc.sync.dma_start(
                ar8[:], ar_out[:].rearrange("(d p) q -> p d q", p=128))
            x2 = []
            for d in range(DT8):
                xt2 = x2p.tile([128, TC], F32R, tag=f"x2_{d}", name=f"x2_{d}")
                nc.vector.scalar_tensor_tensor(
                    xt2[:], ar8[:, d, :], bo_sb[:, d:d + 1], xr8[:, d, :],
                    OP.add, OP.add)
                x2.append(xt2)
            # LN2 stats for own chunk
            sum_ps = lnps.tile([128, TC], F32, tag="s2", name="s2")
            sq_ps = lnps.tile([128, TC], F32, tag="q2", name="q2")
            for d in range(DT8):
                sq = sqp2.tile([128, TC], F32R, tag="sq2", name="sq2")
                nc.scalar.activation(sq[:], x2[d][:], AF.Square)
                nc.tensor.matmul(sum_ps[:], r32(ones_sb[:]),
                                 r32(x2[d][:]),
                                 start=(d == 0), stop=(d == DT8 - 1),
                                 skip_group_check=True)
                nc.tensor.matmul(sq_ps[:], r32(ones_sb[:]), r32(sq[:]),
                                 start=(d == 0), stop=(d == DT8 - 1),
                                 skip_group_check=True)
            mu2 = ln2sb.tile([128, TC], F32, tag="mu2", name="mu2")
            nrsig2 = ln2sb.tile([128, TC], F32, tag="nrsig2", name="nrsig2")
            wrk2 = ln2sb.tile([128, TC], F32, tag="wrk2", name="wrk2")
            nc.vector.tensor_scalar_mul(mu2[:], sum_ps[:], 1.0 / D)
            nc.vector.tensor_scalar_mul(wrk2[:], sq_ps[:], 1.0 / D)
            nc.vector.tensor_tensor(nrsig2[:], mu2[:], mu2[:], OP.mult)
            nc.vector.tensor_sub(wrk2[:], wrk2[:], nrsig2[:])
            nc.vector.tensor_scalar_add(wrk2[:], wrk2[:], LN_EPS)
            nc.scalar.activation(wrk2[:], wrk2[:], AF.Sqrt)
            nc.vector.reciprocal(nrsig2[:], wrk2[:])
            murs2 = ln2sb.tile([128, TC], F32, tag="murs2", name="murs2")
            nc.vector.tensor_tensor(murs2[:], mu2[:], nrsig2[:], OP.mult)
            # x2s = x2 * rsig2 (bf16); mean correction folded via stt + gelu
            x2s = []
            for d in range(DT8):
                xs_ = ln2sb.tile([128, TC], BF16, tag=f"x2s_{d}",
                                 name=f"x2s_{d}")
                nc.vector.tensor_tensor(xs_[:], x2[d][:], nrsig2[:], OP.mult)
                x2s.append(xs_)
            # FFN1: stream w1 tiles by hidden-group; full 4096 hidden.
            # FFN2 for om 0..3 rides along (hm-outer, persistent PSUM
            # accumulators) so it fills PE stalls during FFN1.
            a1 = [a1p.tile([128, TC], BF16, tag=f"a1_{hm}", name=f"a1_{hm}")
                  for hm in range(HM32)]
            facc = [f2ps.tile([128, TC], F32, tag=f"facc{om}",
                              name=f"facc{om}") for om in range(4)]
            for hg in range(FF // 512):
                w1t = w1p.tile([128, DT8, 512], BF16, tag="w1g", name="w1g")
                nc.sync.dma_start(
                    w1t[:], w1[:, hg * 512:(hg + 1) * 512]
                    .rearrange("(d p) h -> p d h", p=128))
                for hl in range(4):
                    hm = hg * 4 + hl
                    hsl = slice(hl * 128, (hl + 1) * 128)
                    ap_ = f1ps.tile([128, TC], F32, tag="a1ps", name="a1ps")
                    for d in range(DT8):
                        nc.tensor.matmul(ap_[:], w1t[:, d, hsl], x2s[d][:],
                                         start=(d == 0), stop=(d == DT8 - 1),
                                         skip_group_check=True)
                    a1n = sqp2.tile([128, TC], F32, tag="a1n", name="a1n")
                    nc.vector.scalar_tensor_tensor(
                        a1n[:], murs2[:], ws1_sb[:, hm:hm + 1], ap_[:],
                        OP.mult, OP.subtract)
                    nc.scalar.activation(a1[hm][:], a1n[:], AF.Gelu,
                                         bias=b1_sb[:, hm:hm + 1], scale=-1.0)
                    for om in range(4):
                        osl = slice(om * 128, (om + 1) * 128)
                        nc.tensor.matmul(facc[om][:],
                                         w2_sb[hm // 4][:, hm % 4, osl],
                                         a1[hm][:],
                                         start=(hm == 0),
                                         stop=(hm == HM32 - 1),
                                         skip_group_check=True)
            xoA = xop.tile([128, 4, TC], F32, tag="xoA", name="xoA")
            for om in range(4):
                nc.vector.tensor_tensor(xoA[:, om, :], facc[om][:],
                                        x2[om][:], OP.add)
            nc.sync.dma_start(
                xout[0:512, :].rearrange("(o p) q -> p o q", p=128), xoA[:])
            # FFN2 second half (om 4..7) after all a1 are ready
            xoB = xop.tile([128, 4, TC], F32, tag="xoB", name="xoB")
            for om in range(4, DT8):
                fp_ = f1ps.tile([128, TC], F32, tag="a1ps", name="a1ps")
                for hm in range(HM32):
                    nc.tensor.matmul(fp_[:], w2_sb[hm // 4][:, hm % 4, osl_o(om)],
                                     a1[hm][:],
                                     start=(hm == 0), stop=(hm == HM32 - 1),
                                     skip_group_check=True)
                nc.vector.tensor_tensor(xoB[:, om - 4, :], fp_[:],
                                        x2[om][:], OP.add)
            nc.sync.dma_start(
                xout[512:1024, :].rearrange("(o p) q -> p o q", p=128),
                xoB[:])
    nc.compile()
    return nc


def host_prep(inputs):
    """Build per-core input maps from the full problem inputs."""
    x = np.asarray(inputs["x"], np.float32)
    mask = np.asarray(inputs["mask"])
    ln1_g = np.asarray(inputs["ln1_g"], np.float32)
    ln1_b = np.asarray(inputs["ln1_b"], np.float32)
    ln2_g = np.asarray(inputs["ln2_g"], np.float32)
    ln2_b = np.asarray(inputs["ln2_b"], np.float32)
    Wq = np.asarray(inputs["Wq"], np.float32)
    Wk = np.asarray(inputs["Wk"], np.float32)
    Wv = np.asarray(inputs["Wv"], np.float32)
    Wo = np.asarray(inputs["Wo"], np.float32)
    bo = np.asarray(inputs["bo"], np.float32)
    W1 = np.asarray(inputs["W1"], np.float32)
    b1 = np.asarray(inputs["b1"], np.float32)
    W2 = np.asarray(inputs["W2"], np.float32)
    b2 = np.asarray(inputs["b2"], np.float32)

    maskT = np.ascontiguousarray(mask.T).astype(np.float32).astype(NPBF16)
    Wq_f = ln1_g[:, None] * Wq
    Wk_f = ln1_g[:, None] * Wk
    Wv_f = ln1_g[:, None] * Wv
    W1_f = ln2_g[:, None] * W1
    w1_bf = W1_f.astype(NPBF16)
    w2_bf = np.ascontiguousarray(W2).astype(NPBF16)
    ws1 = w1_bf.astype(np.float32).sum(0)
    b1_full = ln2_b @ w1_bf.astype(np.float32) + b1
    in_maps = []
    for c in range(NCORES):
        b, g = divmod(c, 4)
        cs = slice(g * C, (g + 1) * C)
        wq_s = np.ascontiguousarray(Wq_f[:, cs])
        wk_s = np.ascontiguousarray(Wk_f[:, cs])
        wv_s = np.ascontiguousarray(Wv_f[:, cs])
        xTb = np.ascontiguousarray(x[b].T)
        wq8 = wq_s.astype(NPBF16)
        wk8 = wk_s.astype(NPBF16)
        wv8 = wv_s.astype(NPBF16)
        scal = np.empty((128, 8 + 2 * C), np.float32)
        scal[:, 0:2] = wq8.astype(np.float32).sum(0).reshape(2, 128).T
        scal[:, 2:4] = wk8.astype(np.float32).sum(0).reshape(2, 128).T
        scal[:, 4:6] = (ln1_b @ Wq[:, cs]).reshape(2, 128).T
        scal[:, 6:8] = (ln1_b @ Wk[:, cs]).reshape(2, 128).T
        scal[:, 8:8 + C] = wv8.astype(np.float32).sum(0)[None, :]
        scal[:, 8 + C:8 + 2 * C] = (ln1_b @ Wv[:, cs])[None, :]
        colb = np.empty((128, DT8 + 2 * HM32), np.float32)
        colb[:, 0:DT8] = bo.reshape(DT8, 128).T
        colb[:, DT8:DT8 + HM32] = b1_full.reshape(HM32, 128).T
        colb[:, DT8 + HM32:] = ws1.reshape(HM32, 128).T
        m = {
            "xT": xTb.astype(NPBF16),
            "xTc": np.ascontiguousarray(xTb[:, g * TC:(g + 1) * TC]),
            "maskT": maskT,
            "wqkv": np.concatenate([wq8, wk8, wv8], axis=1),
            "wo": np.ascontiguousarray(Wo[cs, :]).astype(NPBF16),
            "w1": w1_bf,
            "w2": w2_bf,
            "scal_b": scal,
            "col_b": colb,
        }
        in_maps.append(m)
    return in_maps, b2


def host_assemble(out_maps, b2):
    out = np.empty((B, T, D), np.float32)
    for c in range(NCORES):
        b, g = divmod(c, 4)
        out[b, g * TC:(g + 1) * TC, :] = out_maps[c]["xoutT"].T + b2
    return out


# ======================================================================
# Harness entry point
# ======================================================================
_NC_CACHE = {}


def _get_nc():
    if "nc" not in _NC_CACHE:
        _NC_CACHE["nc"] = build_nc()
    return _NC_CACHE["nc"]


def kernel(**inputs):
    """Full-input / full-output BigBird block on 8 NeuronCores."""
    from concourse.bass_utils import run_bass_kernel_spmd
    nc = _get_nc()
    in_maps, b2 = host_prep(inputs)
    res = run_bass_kernel_spmd(nc, in_maps, list(range(NCORES)))
    return host_assemble(res.results, b2)


# revision 62
# speedup vs baseline: 1.5690x; 1.0703x over previous
"""BigBird block Trainium2 kernel: 8-core SPMD, v2.

Sharding: core c -> batch b = c//4, group rank g = c%4.
  - attention: heads 4g..4g+3 (C=256 qkv cols), tensor-parallel
  - Wo partials ReduceScattered over token chunks: rank g receives the
    group-summed attention output for tokens [512g, 512(g+1))
  - FFN: token-parallel with the FULL 4096 hidden dim per core; each
    core emits the final x2+ff for its own 512 tokens (no second
    collective, no host-side partial summing)

All compute uses feature-major ("transposed") layouts [feature, token] so
matmul contractions keep features on partitions.  LN1 is folded into the
QKV projections (per-token mu/rsig applied post-matmul); LN2 is explicit.
Softmax denominators ride along the attn@V matmul via a ones column
appended to each V tile (65-wide per-head stationary operand).
q/k/attn-out/Wo/W1/W2 run in bf16 (PSUM accumulation stays f32).
"""
import sys
from contextlib import ExitStack

sys.path.insert(0, "/opt/trn_rl_repo")
import numpy as np
import concourse.bacc as bacc
import concourse.mybir as mybir
from concourse import tile

F32 = mybir.dt.float32
F32R = mybir.dt.float32r
BF16 = mybir.dt.bfloat16
FP8 = mybir.dt.float8e4
NPBF16 = mybir.dt.np(BF16)
NPFP8 = mybir.dt.np(FP8)
W8SCALE = 64.0   # fp8 weight pre-scale (keeps 0.02-scale weights normal)

B, T, D, H, HD = 2, 2048, 1024, 16, 64
C = 256          # qkv cols per core (4 heads)
FF = 4096        # full ffn hidden (token-parallel ffn)
TC = 512         # tokens per core after reduce-scatter
NCORES = 8
GROUPS = [[0, 1, 2, 3], [4, 5, 6, 7]]
DT8 = D // 128   # 8 d-tiles
TT16 = T // 128  # 16 token tiles
HM32 = FF // 128  # 32 hidden tiles
LN_EPS = 1e-5

AF = mybir.ActivationFunctionType
OP = mybir.AluOpType


def r32(ap):
    return ap.bitcast(F32R)


def osl_o(om):
    return slice(om * 128, (om + 1) * 128)


def build_nc():
    nc = bacc.Bacc("TRN2", target_bir_lowering=False, debug=False,
                   num_devices=NCORES)
    dt = nc.dram_tensor
    xT = dt("xT", [D, T], BF16, kind="ExternalInput")
    xTc = dt("xTc", [D, TC], F32, kind="ExternalInput")
    maskT = dt("maskT", [T, T], BF16, kind="ExternalInput")
    wqkv = dt("wqkv", [D, 3 * C], BF16, kind="ExternalInput")  # wq|wk|wv
    wo = dt("wo", [C, D], BF16, kind="ExternalInput")
    w1 = dt("w1", [D, FF], BF16, kind="ExternalInput")
    w2 = dt("w2", [FF, D], BF16, kind="ExternalInput")
    # scal blob: wsq|wsk|bq|bk (2 cols each) then wsv_bc|bv_bc (C each)
    scal_b = dt("scal_b", [128, 8 + 2 * C], F32, kind="ExternalInput")
    # col blob: bo (DT8) | b1 (HM32) | ws1 (HM32)
    col_b = dt("col_b", [128, DT8 + 2 * HM32], F32, kind="ExternalInput")

    xout = dt("xoutT", [D, TC], F32, kind="ExternalOutput")
    ar_in = dt("ar_in", [4, D, TC], FP8, kind="Internal")
    ar_out = dt("ar_out", [D, TC], FP8, kind="Internal")

    with ExitStack() as es:
        es.enter_context(nc.allow_low_precision(
            reason="bf16/fp32r SBUF tiles feed the PE; accumulation stays f32"))
        tc = es.enter_context(tile.TileContext(nc))

        def pool(name, bufs, space="SBUF"):
            return tc.tile_pool(name=name, bufs=bufs, space=space)

        pp = es.enter_context(pool("persist", 1))
        ones_sb = pp.tile([128, 128], F32R, name="ones_sb")
        nc.gpsimd.memset(ones_sb[:].bitcast(F32), 1.0)
        ones_sbf = pp.tile([128, 128], BF16, name="ones_sbf")
        nc.gpsimd.memset(ones_sbf[:], 1.0)
        inv128 = pp.tile([128, 1], F32R, name="inv128")
        nc.gpsimd.memset(inv128[:].bitcast(F32), 1.0 / 128.0)
        ones_bf = pp.tile([1, 128], BF16, name="ones_bf")
        nc.gpsimd.memset(ones_bf[:], 1.0)

        # qT/kT/v live from phase 2 through phase 4
        s234 = es.enter_context(ExitStack())
        qsb = s234.enter_context(pool("qkv_sb", 1))

        # ================ phases 1+2: LN1 stats + QKV ====================
        with ExitStack() as s12:
            # qkv weights + scalars first: batched small DMAs that unblock
            # the QKV matmuls; x tiles follow on the shared DMA device.
            wp = s12.enter_context(pool("wqkv", 1))
            wq_sb, wk_sb, wv_sb = [], [], []
            for d in range(DT8):
                w_ = wp.tile([128, 3 * C], BF16, tag=f"wqkv{d}",
                             name=f"wqkv{d}")
                nc.sync.dma_start(w_[:], wqkv[d * 128:(d + 1) * 128, :])
                wq_sb.append(w_[:, 0:C])
                wk_sb.append(w_[:, C:2 * C])
                wv_sb.append(w_[:, 2 * C:3 * C])
            scalt = wp.tile([128, 8 + 2 * C], F32, tag="scal", name="scal")
            nc.sync.dma_start(scalt[:], scal_b[:])
            scal = {"wsq": scalt[:, 0:2], "wsk": scalt[:, 2:4],
                    "bq": scalt[:, 4:6], "bk": scalt[:, 6:8]}
            wsv_sb = scalt[:, 8:8 + C]
            bv_sb = scalt[:, 8 + C:8 + 2 * C]

            xres = s12.enter_context(pool("xres", 1))
            xts = []
            for d in range(DT8):
                t_ = xres.tile([128, T], BF16, tag=f"xt{d}", name=f"xt{d}")
                nc.sync.dma_start(t_[:], xT[d * 128:(d + 1) * 128, :])
                xts.append(t_)

            statsb = s12.enter_context(pool("statsb", 1))
            mu_bc = statsb.tile([128, T], F32, tag="mu", name="mu")
            nrsig_bc = statsb.tile([128, T], F32, tag="nrsig", name="nrsig")
            murs_bc = statsb.tile([128, T], F32, tag="murs", name="murs")
            wrk = statsb.tile([128, T], F32, tag="wrk", name="wrk")

            # LN1 stats, chunked by 512-token groups so downstream work can
            # start as soon as the first chunk's mu/rsig are ready.
            rsig_col, murs_col = [None] * TT16, [None] * TT16
            with pool("sqp", 2) as sqp, pool("statps", 1, "PSUM") as statps:
                sum_ps = [statps.tile([128, 512], F32, tag=f"sum{n}", name=f"sum{n}")
                          for n in range(4)]
                sq_ps = [statps.tile([128, 512], F32, tag=f"sq{n}", name=f"sq{n}")
                         for n in range(4)]
                for d in range(DT8):
                    sq = sqp.tile([128, T], BF16, tag="sq", name="sq")
                    nc.scalar.activation(sq[:], xts[d][:], AF.Square)
                    for n in range(4):
                        sl = slice(n * 512, (n + 1) * 512)
                        nc.tensor.matmul(sum_ps[n][:], ones_sbf[:],
                                         xts[d][:, sl],
                                         start=(d == 0), stop=(d == DT8 - 1),
                                         skip_group_check=True)
                        nc.tensor.matmul(sq_ps[n][:], ones_sbf[:],
                                         sq[:, sl],
                                         start=(d == 0), stop=(d == DT8 - 1),
                                         skip_group_check=True)
                for n in range(4):
                    sl = slice(n * 512, (n + 1) * 512)
                    nc.vector.tensor_scalar_mul(mu_bc[:, sl], sum_ps[n][:],
                                                1.0 / D)
                    nc.vector.tensor_scalar_mul(wrk[:, sl], sq_ps[n][:],
                                                1.0 / D)
                    # var = E[x^2] - mu^2 + eps; rsig = 1/sqrt(var)
                    nc.vector.tensor_tensor(murs_bc[:, sl], mu_bc[:, sl],
                                            mu_bc[:, sl], OP.mult)
                    nc.vector.tensor_sub(wrk[:, sl], wrk[:, sl],
                                         murs_bc[:, sl])
                    nc.vector.tensor_scalar_add(wrk[:, sl], wrk[:, sl],
                                                LN_EPS)
                    nc.scalar.activation(wrk[:, sl], wrk[:, sl], AF.Sqrt)
                    nc.vector.reciprocal(murs_bc[:, sl], wrk[:, sl])  # rsig
                    nc.vector.tensor_scalar_mul(nrsig_bc[:, sl],
                                                murs_bc[:, sl], -1.0)
                    nc.vector.tensor_tensor(murs_bc[:, sl], mu_bc[:, sl],
                                            murs_bc[:, sl], OP.mult)
                    # per-token scalar columns (for the v path); reuse the
                    # consumed stats PSUM banks for the 1-col matmuls
                    for ti, tt in enumerate(range(4 * n, 4 * n + 4)):
                        tsl = slice(tt * 128, (tt + 1) * 128)
                        pr = sum_ps[n][:, ti:ti + 1]
                        nc.tensor.matmul(pr, nrsig_bc[:, tsl],
                                         inv128[:].bitcast(F32),
                                         start=True, stop=True,
                                         skip_group_check=True)
                        rc = statsb.tile([128, 1], F32, tag=f"rc{tt}",
                                         name=f"rc{tt}")
                        nc.vector.tensor_scalar_mul(rc[:], pr, -1.0)
                        rsig_col[tt] = rc
                        pm = sq_ps[n][:, ti:ti + 1]
                        nc.tensor.matmul(pm, murs_bc[:, tsl],
                                         inv128[:].bitcast(F32),
                                         start=True, stop=True,
                                         skip_group_check=True)
                        mc = statsb.tile([128, 1], F32, tag=f"mc{tt}",
                                         name=f"mc{tt}")
                        nc.vector.tensor_copy(mc[:], pm)
                        murs_col[tt] = mc

            # ---- QKV ----
            with pool("qkps", 4, "PSUM") as qkps, pool("qtmp", 2) as qtmp:
                qT, kT = [], []
                for zname, wz, lst, ws_key, b_key in (
                        ("q", wq_sb, qT, "wsq", "bq"),
                        ("k", wk_sb, kT, "wsk", "bk")):
                    for m in range(2):
                        zt = qsb.tile([128, T], BF16, tag=f"{zname}T{m}", name=f"{zname}T{m}")
                        msl = slice(m * 128, (m + 1) * 128)
                        for n in range(4):
                            nsl = slice(n * 512, (n + 1) * 512)
                            zp = qkps.tile([128, 512], F32, tag="zp", name="zp")
                            for d in range(DT8):
                                nc.tensor.matmul(
                                    zp[:], wz[d][:, msl],
                                    xts[d][:, nsl],
                                    start=(d == 0), stop=(d == DT8 - 1),
                                    skip_group_check=True)
                            # (mu*wsz - raw); then z = that*(-rsig) + b
                            # SBUF-only follow-ups alternate onto gpsimd
                            tmpz = qtmp.tile([128, 512], F32, tag="tmpz", name="tmpz")
                            nc.vector.scalar_tensor_tensor(
                                tmpz[:], mu_bc[:, nsl],
                                scal[ws_key][:, m:m + 1], zp[:],
                                OP.mult, OP.subtract)
                            zeng = nc.gpsimd if n % 2 else nc.vector
                            zeng.tensor_tensor(tmpz[:], tmpz[:],
                                               nrsig_bc[:, nsl], OP.mult)
                            zeng.tensor_scalar(zt[:, nsl], tmpz[:],
                                               scal[b_key][:, m:m + 1],
                                               None, OP.add)
                        lst.append(zt)

                # v natural [t-part, c-free] bf16, 65-stride + ones column
                v_sb = []
                for tt in range(TT16):
                    vt = qsb.tile([128, 4 * 65], BF16, tag=f"v{tt}", name=f"v{tt}")
                    v3 = vt[:].rearrange("p (h c) -> p h c", h=4)
                    nc.gpsimd.memset(v3[:, :, 64:65], 1.0)
                    vp = qkps.tile([128, C], F32, tag="vp", name="vp")
                    tsl = slice(tt * 128, (tt + 1) * 128)
                    for d in range(DT8):
                        nc.tensor.matmul(vp[:], xts[d][:, tsl],
                                         wv_sb[d][:],
                                         start=(d == 0), stop=(d == DT8 - 1),
                                         skip_group_check=True)
                    tmp2 = qtmp.tile([128, C], F32, tag="tmp2", name="tmp2")
                    veng = nc.gpsimd if tt % 2 else nc.vector
                    veng.tensor_scalar(tmp2[:], wsv_sb[:],
                                       murs_col[tt][:], None, OP.mult)
                    veng.tensor_sub(tmp2[:], bv_sb[:], tmp2[:])
                    vp3 = vp[:].rearrange("p (h c) -> p h c", h=4)
                    t23 = tmp2[:].rearrange("p (h c) -> p h c", h=4)
                    nc.vector.scalar_tensor_tensor(
                        v3[:, :, 0:64], vp3[:, :, :], rsig_col[tt][:],
                        t23[:, :, :], OP.mult, OP.add)
                    v_sb.append(vt)
        # s12 closed: xts + stats freed; qT/kT/v_sb persist in qsb.

        # ================ phase 3+4: attention, Wo, ReduceScatter ========
        asb = s234.enter_context(pool("att_sb", 1))
        attnT = [asb.tile([128, T], BF16, tag=f"aT{m}", name=f"aT{m}") for m in range(2)]
        den = [asb.tile([1, T], F32, tag=f"den{h}", name=f"den{h}")
               for h in range(4)]
        rden = [asb.tile([1, T], BF16, tag=f"rden{h}", name=f"rden{h}")
                for h in range(4)]
        wo_sb = []
        for cc in range(2):
            w_ = asb.tile([128, D], BF16, tag=f"wo{cc}", name=f"wo{cc}")
            nc.sync.dma_start(w_[:], wo[cc * 128:(cc + 1) * 128, :])
            wo_sb.append(w_)
        with pool("mskp", 1) as mskp, pool("ptp", 4) as ptp, \
             pool("sps", 2, "PSUM") as spsp, \
             pool("avps", 1, "PSUM") as avps, \
             pool("dnps", 1, "PSUM") as dnps, \
             pool("arp", 2) as arp:
            for j in range(4):
                n_kt = 4 * j + 4
                qsl = slice(j * 512, (j + 1) * 512)
                # one batched mask DMA per chunk: [p, kt, q] <- maskT rows
                # (single rotating tag keeps far-future chunks from
                # hoisting their loads ahead of the critical x tiles)
                mj = mskp.tile([128, n_kt, 512], BF16, tag="mj",
                               name=f"mj{j}")
                nc.sync.dma_start(
                    mj[:], maskT[0:n_kt * 128, qsl]
                    .rearrange("(kt p) q -> p kt q", p=128))
                mts = [mj[:, kt, :] for kt in range(n_kt)]
                for hp in range(2):
                    avA = avps.tile([65, 512], F32, tag="avA", name="avA")
                    avB = avps.tile([65, 512], F32, tag="avB", name="avB")
                    for kt in range(n_kt):
                        ksl = slice(kt * 128, (kt + 1) * 128)
                        sps = spsp.tile([128, 1024], F32, tag="sps", name="sps")
                        nc.tensor.matmul(
                            sps[:, 0:512], kT[hp][0:64, ksl],
                            qT[hp][0:64, qsl], start=True, stop=True,
                            tile_position=(0, 0), skip_group_check=True)
                        nc.tensor.matmul(
                            sps[:, 512:1024], kT[hp][64:128, ksl],
                            qT[hp][64:128, qsl], start=True, stop=True,
                            tile_position=(64, 0), skip_group_check=True)
                        pt = ptp.tile([128, 1024], BF16, tag="pt", name="pt")
                        nc.scalar.activation(pt[:], sps[:], AF.Exp,
                                             scale=0.125)
                        # mask multiply: offload 1/3 to the idle gpsimd
                        meng = nc.gpsimd if kt % 3 == 2 else nc.vector
                        meng.tensor_mul(pt[:, 0:512], pt[:, 0:512],
                                        mts[kt][:])
                        meng.tensor_mul(pt[:, 512:1024],
                                        pt[:, 512:1024], mts[kt][:])
                        vv = v_sb[kt][:].rearrange("p (h c) -> p h c", h=4)
                        nc.tensor.matmul(
                            avA[:], vv[:, 2 * hp, :], pt[:, 0:512],
                            start=(kt == 0), stop=(kt == n_kt - 1),
                            skip_group_check=True)
                        nc.tensor.matmul(
                            avB[:], vv[:, 2 * hp + 1, :], pt[:, 512:1024],
                            start=(kt == 0), stop=(kt == n_kt - 1),
                            skip_group_check=True)
                    nc.scalar.copy(attnT[hp][0:64, qsl], avA[0:64, :])
                    nc.scalar.copy(attnT[hp][64:128, qsl], avB[0:64, :])
                    nc.vector.tensor_copy(den[2 * hp][:, qsl], avA[64:65, :])
                    nc.vector.tensor_copy(den[2 * hp + 1][:, qsl],
                                          avB[64:65, :])
                # normalize chunk j and project through Wo, then RS input j
                for h in range(4):
                    nc.vector.reciprocal(rden[h][:, qsl], den[h][:, qsl])
                for m in range(2):
                    dp = dnps.tile([128, 512], F32, tag="dp", name="dp")
                    nc.tensor.matmul(dp[0:64, :],
                                     ones_bf[0:1, 0:64],
                                     rden[2 * m][:, qsl],
                                     start=True, stop=True,
                                     skip_group_check=True)
                    nc.tensor.matmul(dp[64:128, :],
                                     ones_bf[0:1, 64:128],
                                     rden[2 * m + 1][:, qsl],
                                     start=True, stop=True,
                                     skip_group_check=True)
                    nc.vector.tensor_mul(attnT[m][:, qsl], attnT[m][:, qsl],
                                         dp[:])
                aoj = arp.tile([128, DT8, 512], FP8, tag="ao", name="ao")
                for o in range(DT8):
                    osl = slice(o * 128, (o + 1) * 128)
                    wps = dnps.tile([128, 512], F32, tag="wps", name="wps")
                    for cc in range(2):
                        nc.tensor.matmul(
                            wps[:], wo_sb[cc][:, osl],
                            attnT[cc][:, qsl],
                            start=(cc == 0), stop=(cc == 1),
                            skip_group_check=True)
                    nc.vector.tensor_copy(aoj[:, o, :], wps[:])
                nc.sync.dma_start(
                    ar_in[j, :, :].rearrange("(o p) q -> p o q", p=128),
                    aoj[:])
            # one ReduceScatter over token chunks: rank g gets chunk g summed
            nc.gpsimd.collective_compute(
                "ReduceScatter", mybir.AluOpType.add,
                replica_groups=GROUPS,
                ins=[ar_in[:]], outs=[ar_out[:]])
        s234.close()  # free qT/kT/v/attnT SBUF before phases 5-6

        # ========= phases 5+6: own-chunk x2 + LN2 + full-hidden FFN ======
        with pool("x2p", 1) as x2p, pool("ln2sb", 1) as ln2sb, \
             pool("sqp2", 2) as sqp2, pool("arl", 1) as arl, \
             pool("lnps", 1, "PSUM") as lnps, \
             pool("f1ps", 2, "PSUM") as f1ps, \
             pool("f2ps", 1, "PSUM") as f2ps, \
             pool("w1p", 2) as w1p, pool("a1p", 1) as a1p, \
             pool("xop", 1) as xop, pool("colp", 1) as colp, \
             pool("w2p", 1) as w2p:
            colt = colp.tile([128, DT8 + 2 * HM32], F32, tag="colb",
                             name="colb")
            nc.sync.dma_start(colt[:], col_b[:])
            bo_sb = colt[:, 0:DT8]
            b1_sb = colt[:, DT8:DT8 + HM32]
            ws1_sb = colt[:, DT8 + HM32:DT8 + 2 * HM32]
            w2_sb = []
            for h4 in range(HM32 // 4):
                w2t = w2p.tile([128, 4, D], BF16, tag=f"w2_{h4}",
                               name=f"w2_{h4}")
                nc.sync.dma_start(
                    w2t[:], w2[h4 * 512:(h4 + 1) * 512, :]
                    .rearrange("(hh p) o -> p hh o", p=128))
                w2_sb.append(w2t)

            # x2 = x + attn_out + bo for own tokens (batched loads)
            xr8 = arl.tile([128, DT8, TC], F32, tag="xr", name="xr")
            nc.sync.dma_start(
                xr8[:], xTc[:].rearrange("(d p) q -> p d q", p=128))
            ar8 = arl.tile([128, DT8, TC], FP8, tag="art", name="art")
            nc.sync.dma_start(
                ar8[:], ar_out[:].rearrange("(d p) q -> p d q", p=128))
            x2 = []
            for d in range(DT8):
                xt2 = x2p.tile([128, TC], F32R, tag=f"x2_{d}", name=f"x2_{d}")
                nc.vector.scalar_tensor_tensor(
                    xt2[:], ar8[:, d, :], bo_sb[:, d:d + 1], xr8[:, d, :],
                    OP.add, OP.add)
                x2.append(xt2)
            # LN2 stats for own chunk
            sum_ps = lnps.tile([128, TC], F32, tag="s2", name="s2")
            sq_ps = lnps.tile([128, TC], F32, tag="q2", name="q2")
            for d in range(DT8):
                sq = sqp2.tile([128, TC], F32R, tag="sq2", name="sq2")
                nc.scalar.activation(sq[:], x2[d][:], AF.Square)
                nc.tensor.matmul(sum_ps[:], r32(ones_sb[:]),
                                 r32(x2[d][:]),
                                 start=(d == 0), stop=(d == DT8 - 1),
                                 skip_group_check=True)
                nc.tensor.matmul(sq_ps[:], r32(ones_sb[:]), r32(sq[:]),
                                 start=(d == 0), stop=(d == DT8 - 1),
                                 skip_group_check=True)
            mu2 = ln2sb.tile([128, TC], F32, tag="mu2", name="mu2")
            nrsig2 = ln2sb.tile([128, TC], F32, tag="nrsig2", name="nrsig2")
            wrk2 = ln2sb.tile([128, TC], F32, tag="wrk2", name="wrk2")
            nc.vector.tensor_scalar_mul(mu2[:], sum_ps[:], 1.0 / D)
            nc.vector.tensor_scalar_mul(wrk2[:], sq_ps[:], 1.0 / D)
            nc.vector.tensor_tensor(nrsig2[:], mu2[:], mu2[:], OP.mult)
            nc.vector.tensor_sub(wrk2[:], wrk2[:], nrsig2[:])
            nc.vector.tensor_scalar_add(wrk2[:], wrk2[:], LN_EPS)
            nc.scalar.activation(wrk2[:], wrk2[:], AF.Sqrt)
            nc.vector.reciprocal(nrsig2[:], wrk2[:])
            murs2 = ln2sb.tile([128, TC], F32, tag="murs2", name="murs2")
            nc.vector.tensor_tensor(murs2[:], mu2[:], nrsig2[:], OP.mult)
            # x2s = x2 * rsig2 (bf16); mean correction folded via stt + gelu
            x2s = []
            for d in range(DT8):
                xs_ = ln2sb.tile([128, TC], BF16, tag=f"x2s_{d}",
                                 name=f"x2s_{d}")
                nc.vector.tensor_tensor(xs_[:], x2[d][:], nrsig2[:], OP.mult)
                x2s.append(xs_)
            # FFN1: stream w1 tiles by hidden-group; full 4096 hidden.
            # FFN2 for om 0..3 rides along (hm-outer, persistent PSUM
            # accumulators) so it fills PE stalls during FFN1.
            a1 = [a1p.tile([128, TC], BF16, tag=f"a1_{hm}", name=f"a1_{hm}")
                  for hm in range(HM32)]
            facc = [f2ps.tile([128, TC], F32, tag=f"facc{om}",
                              name=f"facc{om}") for om in range(4)]
            for hg in range(FF // 512):
                w1t = w1p.tile([128, DT8, 512], BF16, tag="w1g", name="w1g")
                nc.sync.dma_start(
                    w1t[:], w1[:, hg * 512:(hg + 1) * 512]
                    .rearrange("(d p) h -> p d h", p=128))
                for hl in range(4):
                    hm = hg * 4 + hl
                    hsl = slice(hl * 128, (hl + 1) * 128)
                    ap_ = f1ps.tile([128, TC], F32, tag="a1ps", name="a1ps")
                    for d in range(DT8):
                        nc.tensor.matmul(ap_[:], w1t[:, d, hsl], x2s[d][:],
                                         start=(d == 0), stop=(d == DT8 - 1),
                                         skip_group_check=True)
                    a1n = sqp2.tile([128, TC], F32, tag="a1n", name="a1n")
                    nc.vector.scalar_tensor_tensor(
                        a1n[:], murs2[:], ws1_sb[:, hm:hm + 1], ap_[:],
                        OP.mult, OP.subtract)
                    nc.scalar.activation(a1[hm][:], a1n[:], AF.Gelu,
                                         bias=b1_sb[:, hm:hm + 1], scale=-1.0)
                    for om in range(4):
                        osl = slice(om * 128, (om + 1) * 128)
                        nc.tensor.matmul(facc[om][:],
                                         w2_sb[hm // 4][:, hm % 4, osl],
                                         a1[hm][:],
                                         start=(hm == 0),
                                         stop=(hm == HM32 - 1),
                                         skip_group_check=True)
            xoA = xop.tile([128, 4, TC], F32, tag="xoA", name="xoA")
            for om in range(4):
                nc.vector.tensor_tensor(xoA[:, om, :], facc[om][:],
                                        x2[om][:], OP.add)
            nc.sync.dma_start(
                xout[0:512, :].rearrange("(o p) q -> p o q", p=128), xoA[:])
            # FFN2 second half (om 4..7) after all a1 are ready
            xoB = xop.tile([128, 4, TC], F32, tag="xoB", name="xoB")
            for om in range(4, DT8):
                fp_ = f1ps.tile([128, TC], F32, tag="a1ps", name="a1ps")
                for hm in range(HM32):
                    nc.tensor.matmul(fp_[:], w2_sb[hm // 4][:, hm % 4, osl_o(om)],
                                     a1[hm][:],
                                     start=(hm == 0), stop=(hm == HM32 - 1),
                                     skip_group_check=True)
                nc.vector.tensor_tensor(xoB[:, om - 4, :], fp_[:],
                                        x2[om][:], OP.add)
            nc.sync.dma_start(
                xout[512:1024, :].rearrange("(o p) q -> p o q", p=128),
                xoB[:])
    nc.compile()
    return nc


def host_prep(inputs):
    """Build per-core input maps from the full problem inputs."""
    x = np.asarray(inputs["x"], np.float32)
    mask = np.asarray(inputs["mask"])
    ln1_g = np.asarray(inputs["ln1_g"], np.float32)
    ln1_b = np.asarray(inputs["ln1_b"], np.float32)
    ln2_g = np.asarray(inputs["ln2_g"], np.float32)
    ln2_b = np.asarray(inputs["ln2_b"], np.float32)
    Wq = np.asarray(inputs["Wq"], np.float32)
    Wk = np.asarray(inputs["Wk"], np.float32)
    Wv = np.asarray(inputs["Wv"], np.float32)
    Wo = np.asarray(inputs["Wo"], np.float32)
    bo = np.asarray(inputs["bo"], np.float32)
    W1 = np.asarray(inputs["W1"], np.float32)
    b1 = np.asarray(inputs["b1"], np.float32)
    W2 = np.asarray(inputs["W2"], np.float32)
    b2 = np.asarray(inputs["b2"], np.float32)

    maskT = np.ascontiguousarray(mask.T).astype(np.float32).astype(NPBF16)
    Wq_f = ln1_g[:, None] * Wq
    Wk_f = ln1_g[:, None] * Wk
    Wv_f = ln1_g[:, None] * Wv
    W1_f = ln2_g[:, None] * W1
    w1_bf = W1_f.astype(NPBF16)
    w2_bf = np.ascontiguousarray(W2).astype(NPBF16)
    ws1 = w1_bf.astype(np.float32).sum(0)
    b1_full = ln2_b @ w1_bf.astype(np.float32) + b1
    in_maps = []
    for c in range(NCORES):
        b, g = divmod(c, 4)
        cs = slice(g * C, (g + 1) * C)
        wq_s = np.ascontiguousarray(Wq_f[:, cs])
        wk_s = np.ascontiguousarray(Wk_f[:, cs])
        wv_s = np.ascontiguousarray(Wv_f[:, cs])
        xTb = np.ascontiguousarray(x[b].T)
        wq8 = wq_s.astype(NPBF16)
        wk8 = wk_s.astype(NPBF16)
        wv8 = wv_s.astype(NPBF16)
        scal = np.empty((128, 8 + 2 * C), np.float32)
        scal[:, 0:2] = wq8.astype(np.float32).sum(0).reshape(2, 128).T
        scal[:, 2:4] = wk8.astype(np.float32).sum(0).reshape(2, 128).T
        scal[:, 4:6] = (ln1_b @ Wq[:, cs]).reshape(2, 128).T
        scal[:, 6:8] = (ln1_b @ Wk[:, cs]).reshape(2, 128).T
        scal[:, 8:8 + C] = wv8.astype(np.float32).sum(0)[None, :]
        scal[:, 8 + C:8 + 2 * C] = (ln1_b @ Wv[:, cs])[None, :]
        colb = np.empty((128, DT8 + 2 * HM32), np.float32)
        colb[:, 0:DT8] = bo.reshape(DT8, 128).T
        colb[:, DT8:DT8 + HM32] = b1_full.reshape(HM32, 128).T
        colb[:, DT8 + HM32:] = ws1.reshape(HM32, 128).T
        m = {
            "xT": xTb.astype(NPBF16),
            "xTc": np.ascontiguousarray(xTb[:, g * TC:(g + 1) * TC]),
            "maskT": maskT,
            "wqkv": np.concatenate([wq8, wk8, wv8], axis=1),
            "wo": np.ascontiguousarray(Wo[cs, :]).astype(NPBF16),
            "w1": w1_bf,
            "w2": w2_bf,
            "scal_b": scal,
            "col_b": colb,
        }
        in_maps.append(m)
    return in_maps, b2


def host_assemble(out_maps, b2):
    out = np.empty((B, T, D), np.float32)
    for c in range(NCORES):
        b, g = divmod(c, 4)
        out[b, g * TC:(g + 1) * TC, :] = out_maps[c]["xoutT"].T + b2
    return out


# ======================================================================
# Harness entry point
# ======================================================================
_NC_CACHE = {}


def _get_nc():
    if "nc" not in _NC_CACHE:
        _NC_CACHE["nc"] = build_nc()
    return _NC_CACHE["nc"]


def kernel(**inputs):
    """Full-input / full-output BigBird block on 8 NeuronCores."""
    from concourse.bass_utils import run_bass_kernel_spmd
    nc = _get_nc()
    in_maps, b2 = host_prep(inputs)
    res = run_bass_kernel_spmd(nc, in_maps, list(range(NCORES)))
    return host_assemble(res.results, b2)
